# revision 1
# baseline (speedup 1.0000x reference)
"""Causal self-attention (B=4, T=2048, C=1024, H=16, D=64) on 8 trn2 NeuronCores.

Sharding: core = 2*b + g  (b = batch index 0..3, g = head-group 0..1).
Each core handles one batch and 8 heads (head-dim columns g*512..g*512+512):
  - QKV projection for its slice (tensor parallel over heads, data parallel on B)
  - flash-style causal attention in S^T layout (keys on partitions)
  - partial output projection  z_partial = y_heads @ W_proj[rows of its heads]
Host unshard: z[b] = z_partial[2b] + z_partial[2b+1] + b_proj.

All matmul operands are float32r typed end-to-end (TRN2 single-pass reduced
precision fp32; ~tf32 accuracy) -- the BIR verifier requires producers of
fp32r matmul operands to emit fp32r.

Per-core layout:
  qkTs[tb] [128, 8, 512] : chunks 0-3 = Q^T rows (pre-scaled 1/8), 4-7 = K^T;
                           head h in chunk h//2 (+4 for K), partitions (h%2)*64+
  vnas[tb] [128, 4, 772] : per key-chunk, 4 pair blocks of 193 cols:
                           [v_even(64) | 1 | 1 | zeros(63) | v_odd(64)]
  S^T per (query-block ib of 512, head-pair, key-chunk jb of 128):
      psum[128,2,512] <- two row-tiled K=64 matmuls (heads share the PE array)
      exp on ScalarE (both heads in one [128,1024] activation, PSUM->SBUF)
      causal mask = 0/1 multiply on the 4 diagonal chunks (masks precomputed)
  PV: even head lhsT M=65 [v_e|ones] -> psum rows 0-63 y, row 64 denominator;
      odd head lhsT M=128 window     -> row 0 denominator, rows 64-127 y.
  normalize: reciprocals, pack to partitions 0-1 (tiny DMA), one K=2 selector
      matmul broadcasts them across partitions, multiply -> yTs[ib]
  proj: lhsT = yTs chunks, rhs = W_proj rows, two 512-col blocks -> one DMA out
"""

import sys

import numpy as np

if "/opt/trn_rl_repo" not in sys.path:
    sys.path.insert(0, "/opt/trn_rl_repo")

import concourse.bass as bass
import concourse.bacc as bacc
import concourse.mybir as mybir
import concourse.tile as tile
from concourse.bass_utils import run_bass_kernel_spmd

P = 128
B, C, NH, HD = 4, 1024, 16, 64
T_FULL = 2048
GC = 512          # per-core head-dim columns (8 heads x 64)
TB = 512          # free-dim tile width
NCC = C // P      # 8 contraction chunks for the qkv projection
VB = 193          # vna pair-block width
F32 = mybir.dt.float32
F32R = mybir.dt.float32r
BF16 = mybir.dt.bfloat16

# attention-side operands in bf16 (S/PV matmuls, exp output, masks).
# ~10-20x coarser rounding than float32r; flip only if the accuracy
# budget allows and the speed win is worth it.
import os
ATT_BF16 = bool(int(os.environ.get("ATT_BF16", "0")))

_NC_CACHE = {}


def _build(t_len: int, use_mask: bool, loop_n: int = 0, loop_target: str = 'att') -> bass.Bass:
    from contextlib import ExitStack

    ntb = t_len // TB     # query blocks / t blocks
    AOT = mybir.AluOpType
    ACTF = mybir.ActivationFunctionType

    AT = BF16 if ATT_BF16 else F32R
    nc = bacc.Bacc()
    xT = nc.dram_tensor("xT", [C, t_len], F32R, kind="ExternalInput")
    w_qk = nc.dram_tensor("w_qk", [C, 2 * GC], F32R, kind="ExternalInput")
    w_v = nc.dram_tensor("w_v", [C, GC], F32R, kind="ExternalInput")
    w_pr = nc.dram_tensor("w_pr", [GC, C], F32R, kind="ExternalInput")
    consts = nc.dram_tensor("consts", [P, 640], F32, kind="ExternalInput")
    masks = nc.dram_tensor("masks", [P, 4 * TB], AT, kind="ExternalInput")
    vinit = nc.dram_tensor("vinit", [P, 4 * 4 * VB], AT, kind="ExternalInput")
    bcsel = nc.dram_tensor("bcsel", [P, P], AT, kind="ExternalInput")
    out = nc.dram_tensor("out", [t_len, C], F32, kind="ExternalOutput")

    with tile.TileContext(nc) as tc, ExitStack() as ctx:
        persist = ctx.enter_context(tc.tile_pool(name="persist", bufs=1))
        qkTs = [persist.tile([P, 2 * GC // P, TB], AT, tag=f"qkT{tb}", name=f"qkT{tb}")
                for tb in range(ntb)]
        vnas = [persist.tile([P, TB // P, 4 * VB], AT, tag=f"vna{tb}", name=f"vna{tb}")
                for tb in range(ntb)]
        mk = persist.tile([P, 4, TB], AT)
        cst = persist.tile([P, 640], F32)
        bcs = persist.tile([P, P], AT)
        wpj = persist.tile([P, GC // P, C], F32R)

        nc.sync.dma_start(cst[:], consts[:])
        nc.sync.dma_start(mk[:], masks.rearrange("p (s f) -> p s f", s=4))
        vinit_r = vinit.rearrange("p (j c) -> p j c", j=4)

        xT_r = xT.rearrange("(o p) t -> p o t", p=P)
        wqk_r = w_qk.rearrange("(o p) m -> p o m", p=P)

        # ---------------- Phase 1: QKV projection ----------------
        with tc.tile_pool(name="ph1w", bufs=1) as ph1w, \
             tc.tile_pool(name="ph1", bufs=2) as ph1, \
             tc.tile_pool(name="wqs", bufs=2) as wqs, \
             tc.tile_pool(name="ps1", bufs=3, space="PSUM") as ps1:
            wv_t = ph1w.tile([P, NCC, GC], F32R, tag="wv")
            nc.sync.dma_start(wv_t[:], w_v.rearrange("(o p) n -> p o n", p=P))

            from contextlib import nullcontext
            ph1_loop = (tc.For_i(0, loop_n, 1)
                        if (loop_n and loop_target == "ph1") else nullcontext())
            with ph1_loop:
                for tb in range(ntb):
                  xt = ph1.tile([P, NCC, TB], F32R, tag="xt")
                  nc.sync.dma_start(xt[:], xT_r[:, :, tb * TB:(tb + 1) * TB])
                  nc.sync.dma_start(vnas[tb][:], vinit_r)

                  # V in natural [t, d] layout
                  for tsb in range(TB // P):
                      jc = tb * (TB // P) + tsb
                      ps = ps1.tile([P, TB], F32, tag="ps")
                      for cc in range(NCC):
                          nc.tensor.matmul(
                              ps[:],
                              lhsT=xt[:, cc, tsb * P:(tsb + 1) * P],
                              rhs=wv_t[:, cc, :],
                              start=(cc == 0), stop=(cc == NCC - 1),
                          )
                      vv = vnas[tb][:, tsb, :].rearrange("p (pr c) -> p pr c", c=VB)
                      pr_ps = ps[:].rearrange("p (pr two c) -> p pr two c", two=2, c=64)
                      pr_bv = cst[:, 88:600].rearrange(
                          "p (pr two c) -> p pr two c", two=2, c=64)
                      nc.vector.tensor_tensor(
                          vv[:, :, 0:64], pr_ps[:, :, 0, :], pr_bv[:, :, 0, :], AOT.add
                      )
                      nc.vector.tensor_tensor(
                          vv[:, :, 129:193], pr_ps[:, :, 1, :], pr_bv[:, :, 1, :], AOT.add
                      )
                      if use_mask:
                          nc.vector.tensor_scalar_mul(
                              vnas[tb][:, tsb, :], vnas[tb][:, tsb, :],
                              cst[:, 64 + jc:65 + jc]
                          )

                  # Q^T / K^T rows (transposed layout), two m-blocks per W DMA
                  for mbp in range(GC // P):
                      wq = wqs.tile([P, NCC, 2 * P], F32R, tag="wq")
                      nc.sync.dma_start(
                          wq[:], wqk_r[:, :, mbp * 2 * P:(mbp + 1) * 2 * P])
                      for sub in range(2):
                          mb = 2 * mbp + sub
                          ps = ps1.tile([P, TB], F32, tag="ps")
                          for cc in range(NCC):
                              nc.tensor.matmul(
                                  ps[:],
                                  lhsT=wq[:, cc, sub * P:(sub + 1) * P],
                                  rhs=xt[:, cc, :],
                                  start=(cc == 0), stop=(cc == NCC - 1),
                              )
                          dst = qkTs[tb][:, mb, :]
                          bias = cst[:, 80 + mb:81 + mb]
                          if mb < GC // P:
                              nc.vector.tensor_scalar(
                                  dst, ps[:], bias, 0.125, AOT.add, AOT.mult
                              )
                          else:
                              nc.vector.tensor_scalar(
                                  dst, ps[:], bias, None, AOT.add
                              )

        nc.sync.dma_start(bcs[:], bcsel[:])
        nc.sync.dma_start(wpj[:], w_pr.rearrange("(o p) n -> p o n", p=P))

        # ---------------- Phase 2 + 3: attention, then projection ----------------
        with tc.tile_pool(name="ph2", bufs=1) as ph2:
            yTs = [ph2.tile([P, GC // P, TB], F32R, tag=f"yT{ib}", name=f"yT{ib}")
                   for ib in range(ntb)]
            _phase2(nc, tc, qkTs, vnas, mk, bcs, yTs, t_len, use_mask, loop_n, loop_target)
            _phase3(nc, tc, yTs, wpj, out, t_len,
                    loop_n if loop_target == 'proj' else 0)
    nc.finalize()
    return nc


def _phase2(nc, tc, qkTs, vnas, mk, bcs, yTs, t_len, use_mask, loop_n=0, loop_target='att'):
    ntb = t_len // TB
    AOT = mybir.AluOpType
    ACTF = mybir.ActivationFunctionType
    with tc.tile_pool(name="att", bufs=4) as att, \
         tc.tile_pool(name="rts", bufs=2) as rts, \
         tc.tile_pool(name="sps", bufs=3, space="PSUM") as sps, \
         tc.tile_pool(name="pvs", bufs=1, space="PSUM") as pvs:
        from contextlib import nullcontext
        loop_ctx = (tc.For_i(0, loop_n, 1)
            if (loop_n and loop_target == 'att') else nullcontext())
        with loop_ctx:
            _phase2_body(nc, tc, qkTs, vnas, mk, bcs, yTs, t_len, use_mask,
                         att, rts, sps, pvs)


def _phase2_body(nc, tc, qkTs, vnas, mk, bcs, yTs, t_len, use_mask,
                 att, rts, sps, pvs):
    AT = qkTs[0].dtype
    ntb = t_len // TB
    AOT = mybir.AluOpType
    ACTF = mybir.ActivationFunctionType
    for ib in range(ntb):               # query block
        for pr in range(4):             # head pair: heads (2pr, 2pr+1)
            qc, kc = pr, GC // P + pr
            # per-head PV accumulators, one PSUM bank each:
            #   pve: [y_e rows 0..63 | denom_e row 64]     (lhsT M=65)
            #   pvo: [denom_o row 0 | zeros | y_o 64..127] (lhsT M=128)
            pve = pvs.tile([P, TB], F32, tag="pve")
            pvo = pvs.tile([P, TB], F32, tag="pvo")
            njb = 4 * ib + 4
            for jb in range(njb):
                tbk, jo = jb // 4, jb % 4
                sp = sps.tile([P, 2, TB], F32, tag="sp")
                for e in range(2):
                    po = 64 * e
                    nc.tensor.matmul(
                        sp[:, e, :],
                        lhsT=qkTs[tbk][po:po + 64, kc, jo * P:(jo + 1) * P],
                        rhs=qkTs[ib][po:po + 64, qc, :],
                        start=True, stop=True,
                        tile_position=(po, 0),
                    )
                pt = att.tile([P, 2, TB], AT, tag="pt")
                nc.scalar.activation(pt[:], sp[:], ACTF.Exp)
                s = jb - 4 * ib
                if s >= 0:              # diagonal chunk: causal 0/1 mask
                    for e in range(2):
                        nc.vector.tensor_tensor(
                            pt[:, e, :], pt[:, e, :], mk[:, s, :], AOT.mult
                        )
                nc.tensor.matmul(
                    pve[0:65, :],
                    lhsT=vnas[tbk][:, jo, pr * VB:pr * VB + 65],
                    rhs=pt[:, 0, :],
                    start=(jb == 0), stop=(jb == njb - 1),
                )
                nc.tensor.matmul(
                    pvo[:, :],
                    lhsT=vnas[tbk][:, jo, pr * VB + 65:pr * VB + VB],
                    rhs=pt[:, 1, :],
                    start=(jb == 0), stop=(jb == njb - 1),
                )
            # normalize: reciprocal denominators, pack onto partitions 0-1,
            # one K=2 selector matmul broadcasts across partitions
            rt = rts.tile([P, 2, TB], AT, tag="rt")
            with nc.allow_low_precision(reason="fp32r operand prep"):
                nc.vector.reciprocal(rt[64:65, 0, :], pve[64:65, :])
                nc.vector.reciprocal(rt[0:1, 1, :], pvo[0:1, :])
            nc.sync.dma_start(rt[1:2, 1, :], rt[64:65, 0, :])
            ye = yTs[ib][0:64, pr, :]
            yo = yTs[ib][64:128, pr, :]
            nc.vector.tensor_copy(ye, pve[0:64, :])
            nc.vector.tensor_copy(yo, pvo[64:128, :])
            rb = pvs.tile([P, TB], F32, tag="pve", name="rb")
            nc.tensor.matmul(
                rb[:, :],
                lhsT=bcs[0:2, :],
                rhs=rt[0:2, 1, :],
                start=True, stop=True,
            )
            nc.vector.tensor_tensor(ye, ye, rb[0:64, :], AOT.mult)
            nc.vector.tensor_tensor(yo, yo, rb[64:128, :], AOT.mult)


def _phase3(nc, tc, yTs, wpj, out, t_len, loop_n=0):
    from contextlib import nullcontext
    with tc.tile_pool(name="ps3", bufs=3, space="PSUM") as ps3, \
         tc.tile_pool(name="opl", bufs=3) as opl, \
         (tc.For_i(0, loop_n, 1) if loop_n else nullcontext()):
        for tsb in range(t_len // P):
            ib, to = tsb // 4, tsb % 4
            ot = opl.tile([P, C], F32, tag="ot")
            for nb in range(C // TB):
                ps = ps3.tile([P, TB], F32, tag="po")
                for dc in range(GC // P):
                    nc.tensor.matmul(
                        ps[:],
                        lhsT=yTs[ib][:, dc, to * P:(to + 1) * P],
                        rhs=wpj[:, dc, nb * TB:(nb + 1) * TB],
                        start=(dc == 0), stop=(dc == GC // P - 1),
                    )
                nc.vector.tensor_copy(ot[:, nb * TB:(nb + 1) * TB], ps[:])
            nc.sync.dma_start(out[tsb * P:(tsb + 1) * P, :], ot[:])


def _causal_masks() -> np.ndarray:
    s = np.arange(4)[:, None, None]
    p = np.arange(P)[None, :, None]
    f = np.arange(TB)[None, None, :]
    m = (s * P + p <= f).astype(np.float32)          # [4, 128, 512]
    return np.ascontiguousarray(np.transpose(m, (1, 0, 2)).reshape(P, 4 * TB))


def _make_in_maps(x, W_attn, b_attn, W_proj, attention_mask, t_len):
    import ml_dtypes
    adt = ml_dtypes.bfloat16 if ATT_BF16 else np.float32
    masks_arr = _causal_masks().astype(adt)
    bcsel_arr = np.zeros((P, P), np.float32)
    bcsel_arr[0, 64:128] = 1.0
    bcsel_arr[1, 0:64] = 1.0
    vrow = np.zeros((P, 4 * VB), np.float32)
    for prh in range(4):
        vrow[:, prh * VB + 64] = 1.0
        vrow[:, prh * VB + 65] = 1.0
    vinit = np.ascontiguousarray(np.tile(vrow, (1, 4))).astype(adt)
    in_maps = []
    for core in range(8):
        b, g = core // 2, core % 2
        qcols = slice(g * GC, (g + 1) * GC)
        kcols = slice(C + g * GC, C + (g + 1) * GC)
        vcols = slice(2 * C + g * GC, 2 * C + (g + 1) * GC)

        xTn = np.ascontiguousarray(x[b].T.astype(np.float32))
        w_qk = np.ascontiguousarray(
            np.concatenate([W_attn[:, qcols], W_attn[:, kcols]], axis=1).astype(np.float32)
        )
        w_v = np.ascontiguousarray(W_attn[:, vcols].astype(np.float32))
        w_pr = np.ascontiguousarray(W_proj[g * GC:(g + 1) * GC, :].astype(np.float32))

        cst = np.zeros((P, 640), np.float32)
        cst[:, 0:64] = 1.0
        km = attention_mask[b].astype(np.float32).reshape(t_len // P, P).T
        cst[:, 64:64 + t_len // P] = km
        b_qk = np.concatenate([b_attn[qcols], b_attn[kcols]]).astype(np.float32)
        cst[:, 80:88] = b_qk.reshape(8, P).T
        cst[:, 88:600] = np.broadcast_to(b_attn[vcols].astype(np.float32), (P, GC))

        in_maps.append({
            "xT": xTn, "w_qk": w_qk, "w_v": w_v, "w_pr": w_pr,
            "consts": cst, "masks": masks_arr,
            "bcsel": bcsel_arr.astype(adt),
            "vinit": vinit,
        })
    return in_maps


def _run(x, W_attn, b_attn, W_proj, b_proj, attention_mask, trace=False):
    t_len = x.shape[1]
    use_mask = not bool(np.all(attention_mask != 0))
    key = (t_len, use_mask, ATT_BF16)
    if key not in _NC_CACHE:
        _NC_CACHE[key] = _build(t_len, use_mask)
    nc = _NC_CACHE[key]
    in_maps = _make_in_maps(x, W_attn, b_attn, W_proj, attention_mask, t_len)
    res = run_bass_kernel_spmd(nc, in_maps, list(range(8)), trace=trace)
    outs = [res.results[i]["out"] for i in range(8)]
    bp = b_proj.astype(np.float32)[None, :]
    y = np.stack([outs[2 * b] + outs[2 * b + 1] + bp for b in range(B)]).astype(np.float32)
    return y, res


def kernel(x, W_attn, b_attn, W_proj, b_proj, attention_mask):
    x = np.asarray(x, np.float32)
    W_attn = np.asarray(W_attn, np.float32)
    b_attn = np.asarray(b_attn, np.float32)
    W_proj = np.asarray(W_proj, np.float32)
    b_proj = np.asarray(b_proj, np.float32)
    attention_mask = np.asarray(attention_mask)
    y, _ = _run(x, W_attn, b_attn, W_proj, b_proj, attention_mask)
    return y

